# revision 8
# baseline (speedup 1.0000x reference)
"""Deformable Conv2d Lite (K=3) on 8 Trainium2 NeuronCores.

Sharding: data-parallel over batch x image-half. Core n handles sample n//2,
image rows [64*(n%2), 64*(n%2)+64). Weight replicated.

Device pipeline per core:
  1. DVE: from raw offsets compute, per (tap, pixel): a gather index into an
     interleaved row-pair NHWC layout of x, plus 4 bilinear corner weights
     (eq-masked so clamping/out-of-image gives exact zero-padding semantics).
  2. SWDGE dma_gather: one 1024B descriptor per (tap, pixel) fetches the full
     2x2 x 64ch bilinear patch from DRAM.
  3. DVE: weighted 4->1 combine in pixel-major layout (weights broadcast
     along channels via stride-0 AP).
  4. PE: transpose combined samples to channel-major with taps packed in
     pairs (K=128), then 5 accumulated matmuls against the 64x128 weight
     slabs -> PSUM.
  5. ACT: PSUM->SBUF with bias add; DMA out.
"""

import sys

for _p in ("/opt/trn_rl_repo",):
    if _p not in sys.path:
        sys.path.insert(0, _p)

import numpy as np

import concourse.bass as bass
import concourse.tile as tile
from concourse import bacc, mybir
from concourse.bass_utils import run_bass_kernel_spmd

F32 = mybir.dt.float32
I16 = mybir.dt.int16
Alu = mybir.AluOpType
Act = mybir.ActivationFunctionType

B, C, H, W = 4, 64, 128, 128
OC, KK = 64, 9
HALF = H // 2            # rows per core
PIX = HALF * W           # 8192 pixels per core
NCHUNK = 4
CPIX = PIX // NCHUNK     # 2048 pixels per chunk
CSLOT = CPIX // 128      # 16 slots per chunk
SLOTS = PIX // 128       # 64
NPAIR = 5                # ceil(9/2) tap pairs

XROWS = 2 * PIX + 1      # interleaved pair-row count incl. pad


def build_program(loop_n: int = 0):
    """Build the per-core Bass program. loop_n>0 wraps the body in a device
    For_i loop (for wall-clock timing); loop_n==0 emits the plain body."""
    nc = bacc.Bacc("TRN2", target_bir_lowering=False, debug=False, num_devices=8,
                   num_swdge_queues=4)

    xp = nc.dram_tensor("xpair", [XROWS, 128], F32, kind="ExternalInput").ap()
    offs = nc.dram_tensor("offs", [PIX, 2 * KK], F32, kind="ExternalInput").ap()
    wp = nc.dram_tensor("wpair", [128, NPAIR * OC], F32, kind="ExternalInput").ap()
    yyd = nc.dram_tensor("yy", [128, SLOTS], F32, kind="ExternalInput").ap()
    xxd = nc.dram_tensor("xx", [128, 1], F32, kind="ExternalInput").ap()
    idd = nc.dram_tensor("ident", [128, 128], F32, kind="ExternalInput").ap()
    bsd = nc.dram_tensor("bias", [OC, 1], F32, kind="ExternalInput").ap()
    out = nc.dram_tensor("out", [OC, PIX], F32, kind="ExternalOutput").ap()

    with tile.TileContext(nc) as tc:
        import contextlib

        with contextlib.ExitStack() as ctx:
            cpool = ctx.enter_context(tc.tile_pool(name="consts", bufs=1))
            apool = ctx.enter_context(tc.tile_pool(name="stageA", bufs=1))
            gpool = ctx.enter_context(tc.tile_pool(name="gather", bufs=3))
            spool = ctx.enter_context(tc.tile_pool(name="spair", bufs=2))
            stpool = ctx.enter_context(tc.tile_pool(name="stmaj", bufs=3))
            opool = ctx.enter_context(tc.tile_pool(name="outsb", bufs=2))
            ptpool = ctx.enter_context(
                tc.tile_pool(name="psumT", bufs=2, space="PSUM")
            )
            pcpool = ctx.enter_context(
                tc.tile_pool(name="psumC", bufs=1, space="PSUM")
            )

            # ---- constants -------------------------------------------------
            xx = cpool.tile([128, 1], F32)
            nc.sync.dma_start(xx[:], xxd[:, :])
            yy = cpool.tile([128, SLOTS], F32)
            nc.sync.dma_start(yy[:], yyd[:, :])
            ident = cpool.tile([128, 128], F32)
            nc.sync.dma_start(ident[:], idd[:, :])
            wpt = cpool.tile([128, NPAIR * OC], F32)
            nc.sync.dma_start(wpt[:], wp[:, :])
            bias = cpool.tile([OC, 1], F32)
            nc.sync.dma_start(bias[:], bsd[:, :])

            def body(_iv=None):
                # ---- stage A: indices + weights ---------------------------
                # layout [128 part = pixel%128 (img col), slot = pixel//128
                # (img row), tap]
                OFF = apool.tile([128, SLOTS, 2 * KK], F32, name="OFF")
                nc.sync.dma_start(
                    OFF[:], offs.rearrange("(s p) c -> p s c", p=128)
                )
                offx = OFF[:, :, 0 : 2 * KK : 2]   # [128, 64, 9]
                offy = OFF[:, :, 1 : 2 * KK : 2]

                shp = [128, SLOTS, KK]

                def atile(name):
                    return apool.tile(shp, F32, name=name)

                vec = nc.vector
                px = atile("px")
                vec.tensor_scalar(px[:], offx, xx[:, 0:1], None, Alu.add)
                py = atile("py")
                yyb = yy[:, :].unsqueeze(2).broadcast_to(shp)
                vec.tensor_tensor(py[:], offy, yyb, Alu.add)
                # floor via int cast: F = round-ish(v) - (round-ish(v) > v)
                # exact for truncate or round-to-nearest cast semantics.
                casti = apool.tile(shp, mybir.dt.int32, name="casti")
                rnd = atile("rnd")
                tn = atile("tn")

                def floor_into(dst, v):
                    # dst <- floor(v)
                    vec.tensor_copy(casti[:], v[:])
                    vec.tensor_copy(rnd[:], casti[:])
                    vec.tensor_tensor(tn[:], rnd[:], v[:], Alu.is_gt)
                    vec.scalar_tensor_tensor(
                        dst[:], tn[:], -1.0, rnd[:], Alu.mult, Alu.add
                    )

                x0 = atile("x0")
                floor_into(x0, px)
                fx = atile("fx")
                vec.tensor_tensor(fx[:], px[:], x0[:], Alu.subtract)
                y0 = atile("y0")
                floor_into(y0, py)
                fy = atile("fy")
                vec.tensor_tensor(fy[:], py[:], y0[:], Alu.subtract)
                xc = atile("xc")
                vec.tensor_scalar(xc[:], x0[:], 0.0, 126.0, Alu.max, Alu.min)
                dx = atile("dx")
                vec.tensor_tensor(dx[:], xc[:], x0[:], Alu.subtract)
                g0 = atile("g0")
                vec.tensor_scalar(g0[:], y0[:], 0.0, 126.0, Alu.max, Alu.min)
                dy = atile("dy")
                vec.tensor_tensor(dy[:], g0[:], y0[:], Alu.subtract)
                # parity of g0 (integer >= 0): par = g0 - 2*floor(g0/2)
                par = atile("par")
                gh = atile("gh")
                vec.tensor_scalar(gh[:], g0[:], 0.5, None, Alu.mult)
                fl2 = atile("fl2")
                floor_into(fl2, gh)
                vec.scalar_tensor_tensor(
                    par[:], fl2[:], -2.0, g0[:], Alu.mult, Alu.add
                )
                # idx = g0*64 + par*8128 + xc
                idxf = atile("idxf")
                vec.tensor_scalar(idxf[:], g0[:], 64.0, None, Alu.mult)
                vec.tensor_scalar(par[:], par[:], 8128.0, None, Alu.mult)
                vec.tensor_tensor(idxf[:], idxf[:], par[:], Alu.add)
                vec.tensor_tensor(idxf[:], idxf[:], xc[:], Alu.add)
                idx16 = apool.tile([128, KK, SLOTS], I16, name="idx16")
                vec.tensor_copy(
                    idx16[:].rearrange("p t s -> p s t"), idxf[:]
                )

                # weights. wx1=fx, wx0=1-fx
                wx0 = atile("wx0")
                vec.tensor_scalar(wx0[:], fx[:], -1.0, 1.0, Alu.mult, Alu.add)
                wy0 = atile("wy0")
                vec.tensor_scalar(wy0[:], fy[:], -1.0, 1.0, Alu.mult, Alu.add)
                e0 = atile("e0")
                vec.tensor_scalar(e0[:], dx[:], 0.0, None, Alu.is_equal)
                e1 = atile("e1")
                vec.tensor_scalar(e1[:], dx[:], 1.0, None, Alu.is_equal)
                em = atile("em")
                vec.tensor_scalar(em[:], dx[:], -1.0, None, Alu.is_equal)
                # wcL = wx0*e0 + fx*e1 ; wcR = wx0*em + fx*e0
                wcl = atile("wcl")
                vec.tensor_tensor(wcl[:], wx0[:], e0[:], Alu.mult)
                vec.tensor_tensor(e1[:], fx[:], e1[:], Alu.mult)
                vec.tensor_tensor(wcl[:], wcl[:], e1[:], Alu.add)
                wcr = atile("wcr")
                vec.tensor_tensor(wcr[:], wx0[:], em[:], Alu.mult)
                vec.tensor_tensor(e0[:], fx[:], e0[:], Alu.mult)
                vec.tensor_tensor(wcr[:], wcr[:], e0[:], Alu.add)
                # rows
                r0 = atile("r0")
                vec.tensor_scalar(r0[:], dy[:], 0.0, None, Alu.is_equal)
                r1 = atile("r1")
                vec.tensor_scalar(r1[:], dy[:], 1.0, None, Alu.is_equal)
                rm = atile("rm")
                vec.tensor_scalar(rm[:], dy[:], -1.0, None, Alu.is_equal)
                wrt = atile("wrt")
                vec.tensor_tensor(wrt[:], wy0[:], r0[:], Alu.mult)
                vec.tensor_tensor(r1[:], fy[:], r1[:], Alu.mult)
                vec.tensor_tensor(wrt[:], wrt[:], r1[:], Alu.add)
                wrb = atile("wrb")
                vec.tensor_tensor(wrb[:], wy0[:], rm[:], Alu.mult)
                vec.tensor_tensor(r0[:], fy[:], r0[:], Alu.mult)
                vec.tensor_tensor(wrb[:], wrb[:], r0[:], Alu.add)
                # W4 [128, slot, tap, n] with n = (col, row):
                # 0=(L,T) 1=(L,B) 2=(R,T) 3=(R,B)
                w4 = apool.tile([128, SLOTS, KK, 4], F32, name="w4")
                vec.tensor_tensor(w4[:, :, :, 0], wcl[:], wrt[:], Alu.mult)
                vec.tensor_tensor(w4[:, :, :, 1], wcl[:], wrb[:], Alu.mult)
                vec.tensor_tensor(w4[:, :, :, 2], wcr[:], wrt[:], Alu.mult)
                vec.tensor_tensor(w4[:, :, :, 3], wcr[:], wrb[:], Alu.mult)

                # ---- idx shuffle into SWDGE wrapped-16 layout --------------
                # IDXW[q, t, s*8 + k] = idx16[16k+q, s, t]; replicated to all
                # eight 16-partition blocks.
                idxw = apool.tile([128, KK, SLOTS * 8], I16, name="idxw")
                idxwv = idxw[:].rearrange("p t (s k) -> p t s k", k=8)
                for k in range(8):
                    nc.sync.dma_start(
                        idxwv[0:16, :, :, k].squeeze(),
                        idx16[16 * k : 16 * (k + 1), :, :],
                    )
                for blk in range(1, 8):
                    nc.sync.dma_start(
                        idxw[16 * blk : 16 * (blk + 1), :, :], idxw[0:16, :, :]
                    )

                # gather source view: row r -> 256 contiguous floats starting
                # at r*128 (overlapping windows)
                xsrc = bass.AP(xp.tensor, 0, [[128, 2 * PIX], [1, 256]])
                gidx = [0]

                # ---- main loop --------------------------------------------
                for ch in range(NCHUNK):
                    spair = None
                    for t in range(KK):
                        sub = t % 2
                        pair = t // 2
                        g = gpool.tile([128, CSLOT, 4, 64], F32, name="g")
                        for q2 in range(4):
                            nc.gpsimd.dma_gather(
                                g[:, 4 * q2 : 4 * (q2 + 1), :, :].rearrange(
                                    "p s a c -> p s (a c)"
                                ),
                                xsrc,
                                idxw[
                                    :, t,
                                    128 * ch + 32 * q2 : 128 * ch + 32 * (q2 + 1),
                                ],
                                num_idxs=512,
                                num_idxs_reg=512,
                                elem_size=256,
                                elem_step=128,
                                queue_num=gidx[0] % 4,
                            )
                            gidx[0] += 1
                        # combine: g *= w4 (bcast over c); pairwise adds
                        wsl = (
                            w4[:, ch * CSLOT : (ch + 1) * CSLOT, t, :]
                            .unsqueeze(3)
                            .broadcast_to([128, CSLOT, 4, 64])
                        )
                        vec.tensor_tensor(g[:], g[:], wsl, Alu.mult)
                        # n=(col,row): vertical add pairs (0,1) and (2,3)
                        vec.tensor_tensor(
                            g[:, :, 0:4:2, :],
                            g[:, :, 0:4:2, :],
                            g[:, :, 1:4:2, :],
                            Alu.add,
                        )
                        if sub == 0:
                            spair = spool.tile(
                                [128, CSLOT, 2, 64], F32, name="spair"
                            )
                            if t == KK - 1:
                                vec.memset(spair[:, :, 1, :], 0.0)
                        vec.tensor_tensor(
                            spair[:, :, sub, :],
                            g[:, :, 0, :],
                            g[:, :, 2, :],
                            Alu.add,
                        )
                        if sub == 1 or t == KK - 1:
                            # transpose pair -> channel-major, conv matmuls
                            stm = stpool.tile([128, CPIX], F32, name="stm")
                            for tb in range(CSLOT // 4):
                                pt = ptpool.tile([128, 512], F32, name="pt")
                                for j in range(4):
                                    slot = tb * 4 + j
                                    nc.tensor.matmul(
                                        pt[:, 128 * j : 128 * (j + 1)],
                                        spair[:, slot, :, :],
                                        ident[:],
                                        is_transpose=True,
                                    )
                                nc.scalar.activation(
                                    stm[:, 512 * tb : 512 * (tb + 1)],
                                    pt[:],
                                    Act.Copy,
                                )
                            if pair == 0:
                                pc = pcpool.tile([OC, CPIX], F32, name="pc")
                            for nb in range(CPIX // 512):
                                nc.tensor.matmul(
                                    pc[:, 512 * nb : 512 * (nb + 1)],
                                    wpt[:, OC * pair : OC * (pair + 1)],
                                    stm[:, 512 * nb : 512 * (nb + 1)],
                                    start=(pair == 0),
                                    stop=(pair == NPAIR - 1),
                                )
                    osb = opool.tile([OC, CPIX], F32, name="osb")
                    nc.scalar.activation(
                        osb[:], pc[:], Act.Identity, bias=bias[:, 0:1]
                    )
                    nc.sync.dma_start(out[:, CPIX * ch : CPIX * (ch + 1)], osb[:])

            if loop_n > 0:
                with tc.For_i(0, loop_n, 1):
                    body()
            else:
                body()

    nc.compile()
    return nc


def prep_core_inputs(x, offset, weight, bias, core):
    """Host-side shard/layout prep for one core. Pure layout, no math on
    tensor values (beyond the reference-mandated reshape semantics)."""
    s, half = core // 2, core % 2
    # interleaved row-pair NHWC: xpair[par*PIX + pr*128 + j] =
    #   [x[2pr+par, j, :], x[2pr+par+1, j, :]]
    xr = np.ascontiguousarray(x[s].transpose(1, 2, 0))          # [H, W, C]
    xpad = np.concatenate([xr, np.zeros((1, W, C), np.float32)], 0)  # [129,W,C]
    even = np.stack([xpad[0::2][:64], xpad[1::2][:64]], 1)   # rows (2p, 2p+1)
    odd = np.stack([xpad[1::2][:64], xpad[2::2][:64]], 1)    # rows (2p+1, 2p+2)
    rows = np.concatenate([even, odd], 0)                    # [128, 2, W, C]
    # xpair row (par, pr, j) = [x[row_g0, j, :], x[row_g0+1, j, :]]
    xpair = np.concatenate(
        [rows.transpose(0, 2, 1, 3).reshape(2 * PIX, 128),
         np.zeros((1, 128), np.float32)], 0)

    # offsets for this half, pixel-major [PIX, 18]
    off = np.ascontiguousarray(
        offset[s, :, 64 * half : 64 * half + HALF, :]
        .transpose(1, 2, 0)
        .reshape(PIX, 2 * KK)
    )
    # channel c=2t is x-offset, 2t+1 is y-offset (reference reshape
    # [kk,2,h,w]: x = off[:,:,0], y = off[:,:,1] -> channel t*2+0 / t*2+1)

    # tap-pair weight slabs [128, 5*64]: rows sub*64+c, cols pair*64+o
    wfull = weight.reshape(OC, C, KK)
    wpair = np.zeros((128, NPAIR * OC), np.float32)
    for pair in range(NPAIR):
        for sub in range(2):
            t = pair * 2 + sub
            if t < KK:
                wpair[sub * 64 : sub * 64 + 64, pair * OC : (pair + 1) * OC] = (
                    wfull[:, :, t].T
                )

    yy = np.broadcast_to(
        (np.arange(SLOTS, dtype=np.float32) + 64 * half)[None, :], (128, SLOTS)
    ).copy()
    xxc = np.arange(128, dtype=np.float32).reshape(128, 1).copy()
    return {
        "xpair": np.ascontiguousarray(xpair),
        "offs": off.astype(np.float32),
        "wpair": wpair,
        "yy": yy,
        "xx": xxc,
        "ident": np.eye(128, dtype=np.float32),
        "bias": bias.reshape(OC, 1).astype(np.float32),
    }


_CACHE = {}


def kernel(x, offset, weight, bias):
    x = np.asarray(x, np.float32)
    offset = np.asarray(offset, np.float32)
    weight = np.asarray(weight, np.float32)
    bias = np.asarray(bias, np.float32)
    if "nc" not in _CACHE:
        _CACHE["nc"] = build_program()
    nc = _CACHE["nc"]
    in_maps = [prep_core_inputs(x, offset, weight, bias, c) for c in range(8)]
    res = run_bass_kernel_spmd(nc, in_maps, core_ids=list(range(8)))
    outf = np.empty((B, OC, H, W), np.float32)
    for c in range(8):
        s, half = c // 2, c % 2
        outf[s, :, 64 * half : 64 * half + HALF, :] = res.results[c][
            "out"
        ].reshape(OC, HALF, W)
    return outf


# revision 16
# speedup vs baseline: 1.1170x; 1.1170x over previous
"""Deformable Conv2d Lite (K=3) on 8 Trainium2 NeuronCores.

Sharding: data-parallel over batch x image-half. Core n handles sample n//2,
image rows [64*(n%2), 64*(n%2)+64). Weight replicated.

Device pipeline per core:
  1. DVE: from raw offsets compute, per (tap, pixel): a gather index into an
     interleaved row-pair NHWC layout of x, plus 4 bilinear corner weights
     (eq-masked so clamping/out-of-image gives exact zero-padding semantics).
  2. SWDGE dma_gather: one 1024B descriptor per (tap, pixel) fetches the full
     2x2 x 64ch bilinear patch from DRAM.
  3. DVE: weighted 4->1 combine in pixel-major layout (weights broadcast
     along channels via stride-0 AP).
  4. PE: transpose combined samples to channel-major with taps packed in
     pairs (K=128), then 5 accumulated matmuls against the 64x128 weight
     slabs -> PSUM.
  5. ACT: PSUM->SBUF with bias add; DMA out.
"""

import sys

for _p in ("/opt/trn_rl_repo",):
    if _p not in sys.path:
        sys.path.insert(0, _p)

import numpy as np

import concourse.bass as bass
import concourse.tile as tile
from concourse import bacc, mybir
from concourse.bass_utils import run_bass_kernel_spmd

F32 = mybir.dt.float32
F16 = mybir.dt.float16
I16 = mybir.dt.int16
Alu = mybir.AluOpType
Act = mybir.ActivationFunctionType

B, C, H, W = 4, 64, 128, 128
OC, KK = 64, 9
HALF = H // 2            # rows per core
PIX = HALF * W           # 8192 pixels per core
NCHUNK = 4
CPIX = PIX // NCHUNK     # 2048 pixels per chunk
CSLOT = CPIX // 128      # 16 slots per chunk
SLOTS = PIX // 128       # 64
NPAIR = 5                # ceil(9/2) tap pairs

XROWS = 2 * PIX + 1      # interleaved pair-row count incl. pad


def build_program(loop_n: int = 0, ablate: str = ""):
    """Build the per-core Bass program. loop_n>0 wraps the body in a device
    For_i loop (for wall-clock timing); loop_n==0 emits the plain body.
    ablate: comma-set of {nogather, nodve, nope} for perf bisection."""
    abl = set(ablate.split(",")) if ablate else set()
    nc = bacc.Bacc("TRN2", target_bir_lowering=False, debug=False, num_devices=8,
                   num_swdge_queues=4)

    xp = nc.dram_tensor("xpair", [XROWS, 128], F16, kind="ExternalInput").ap()
    offs = nc.dram_tensor("offs", [PIX, 2 * KK], F32, kind="ExternalInput").ap()
    wp = nc.dram_tensor("wpair", [128, NPAIR * OC], F16, kind="ExternalInput").ap()
    yyd = nc.dram_tensor("yy", [128, SLOTS], F32, kind="ExternalInput").ap()
    xxd = nc.dram_tensor("xx", [128, 1], F32, kind="ExternalInput").ap()
    idd = nc.dram_tensor("ident", [128, 128], F16, kind="ExternalInput").ap()
    bsd = nc.dram_tensor("bias", [OC, 1], F32, kind="ExternalInput").ap()
    out = nc.dram_tensor("out", [OC, PIX], F32, kind="ExternalOutput").ap()

    with tile.TileContext(nc) as tc:
        import contextlib

        with contextlib.ExitStack() as ctx:
            cpool = ctx.enter_context(tc.tile_pool(name="consts", bufs=1))
            apool = ctx.enter_context(tc.tile_pool(name="stageA", bufs=1))
            gpool = ctx.enter_context(tc.tile_pool(name="gather", bufs=3))
            spool = ctx.enter_context(tc.tile_pool(name="spair", bufs=2))
            stpool = ctx.enter_context(tc.tile_pool(name="stmaj", bufs=3))
            opool = ctx.enter_context(tc.tile_pool(name="outsb", bufs=2))
            ptpool = ctx.enter_context(
                tc.tile_pool(name="psumT", bufs=2, space="PSUM")
            )
            pcpool = ctx.enter_context(
                tc.tile_pool(name="psumC", bufs=1, space="PSUM")
            )

            # ---- constants -------------------------------------------------
            xx = cpool.tile([128, 1], F32)
            nc.sync.dma_start(xx[:], xxd[:, :])
            yy = cpool.tile([128, SLOTS], F32)
            nc.sync.dma_start(yy[:], yyd[:, :])
            ident = cpool.tile([128, 128], F16)
            nc.sync.dma_start(ident[:], idd[:, :])
            wpt = cpool.tile([128, NPAIR * OC], F16)
            nc.sync.dma_start(wpt[:], wp[:, :])
            bias = cpool.tile([OC, 1], F32)
            nc.sync.dma_start(bias[:], bsd[:, :])

            def body(_iv=None):
                if "noa" in abl:
                    # gather-only isolation: iota indices, no stage A
                    idxw = apool.tile([128, KK, SLOTS * 8], I16, name="idxw")
                    nc.gpsimd.iota(
                        idxw[:].rearrange("p a b -> p (a b)"),
                        pattern=[[3, KK * SLOTS * 8]],
                        base=0,
                        channel_multiplier=0,
                    )
                    w4 = None
                else:
                    w4, idxw = stage_a()
                main_loops(w4, idxw)

            def stage_a():
                # ---- stage A: indices + weights ---------------------------
                # layout [128 part = pixel%128 (img col), slot = pixel//128
                # (img row), tap]
                OFF = apool.tile([128, SLOTS, 2 * KK], F32, name="OFF")
                nc.sync.dma_start(
                    OFF[:], offs.rearrange("(s p) c -> p s c", p=128)
                )
                offx = OFF[:, :, 0 : 2 * KK : 2]   # [128, 64, 9]
                offy = OFF[:, :, 1 : 2 * KK : 2]

                shp = [128, SLOTS, KK]

                def atile(name):
                    return apool.tile(shp, F32, name=name)

                vec = nc.vector
                px = atile("px")
                vec.tensor_scalar(px[:], offx, xx[:, 0:1], None, Alu.add)
                py = atile("py")
                yyb = yy[:, :].unsqueeze(2).broadcast_to(shp)
                vec.tensor_tensor(py[:], offy, yyb, Alu.add)
                # floor via int cast: F = round-ish(v) - (round-ish(v) > v)
                # exact for truncate or round-to-nearest cast semantics.
                casti = apool.tile(shp, mybir.dt.int32, name="casti")
                rnd = atile("rnd")
                tn = atile("tn")

                def floor_into(dst, v):
                    # dst <- floor(v)
                    vec.tensor_copy(casti[:], v[:])
                    vec.tensor_copy(rnd[:], casti[:])
                    vec.tensor_tensor(tn[:], rnd[:], v[:], Alu.is_gt)
                    vec.scalar_tensor_tensor(
                        dst[:], tn[:], -1.0, rnd[:], Alu.mult, Alu.add
                    )

                x0 = atile("x0")
                floor_into(x0, px)
                fx = atile("fx")
                vec.tensor_tensor(fx[:], px[:], x0[:], Alu.subtract)
                y0 = atile("y0")
                floor_into(y0, py)
                fy = atile("fy")
                vec.tensor_tensor(fy[:], py[:], y0[:], Alu.subtract)
                xc = atile("xc")
                vec.tensor_scalar(xc[:], x0[:], 0.0, 126.0, Alu.max, Alu.min)
                dx = atile("dx")
                vec.tensor_tensor(dx[:], xc[:], x0[:], Alu.subtract)
                g0 = atile("g0")
                vec.tensor_scalar(g0[:], y0[:], 0.0, 126.0, Alu.max, Alu.min)
                dy = atile("dy")
                vec.tensor_tensor(dy[:], g0[:], y0[:], Alu.subtract)
                # parity of g0 (integer >= 0): par = g0 - 2*floor(g0/2)
                par = atile("par")
                gh = atile("gh")
                vec.tensor_scalar(gh[:], g0[:], 0.5, None, Alu.mult)
                fl2 = atile("fl2")
                floor_into(fl2, gh)
                vec.scalar_tensor_tensor(
                    par[:], fl2[:], -2.0, g0[:], Alu.mult, Alu.add
                )
                # idx = g0*64 + par*8128 + xc
                idxf = atile("idxf")
                vec.tensor_scalar(idxf[:], g0[:], 64.0, None, Alu.mult)
                vec.tensor_scalar(par[:], par[:], 8128.0, None, Alu.mult)
                vec.tensor_tensor(idxf[:], idxf[:], par[:], Alu.add)
                vec.tensor_tensor(idxf[:], idxf[:], xc[:], Alu.add)
                idx16 = apool.tile([128, KK, SLOTS], I16, name="idx16")
                vec.tensor_copy(
                    idx16[:].rearrange("p t s -> p s t"), idxf[:]
                )

                # weights. wx1=fx, wx0=1-fx
                wx0 = atile("wx0")
                vec.tensor_scalar(wx0[:], fx[:], -1.0, 1.0, Alu.mult, Alu.add)
                wy0 = atile("wy0")
                vec.tensor_scalar(wy0[:], fy[:], -1.0, 1.0, Alu.mult, Alu.add)
                e0 = atile("e0")
                vec.tensor_scalar(e0[:], dx[:], 0.0, None, Alu.is_equal)
                e1 = atile("e1")
                vec.tensor_scalar(e1[:], dx[:], 1.0, None, Alu.is_equal)
                em = atile("em")
                vec.tensor_scalar(em[:], dx[:], -1.0, None, Alu.is_equal)
                # wcL = wx0*e0 + fx*e1 ; wcR = wx0*em + fx*e0
                wcl = atile("wcl")
                vec.tensor_tensor(wcl[:], wx0[:], e0[:], Alu.mult)
                vec.tensor_tensor(e1[:], fx[:], e1[:], Alu.mult)
                vec.tensor_tensor(wcl[:], wcl[:], e1[:], Alu.add)
                wcr = atile("wcr")
                vec.tensor_tensor(wcr[:], wx0[:], em[:], Alu.mult)
                vec.tensor_tensor(e0[:], fx[:], e0[:], Alu.mult)
                vec.tensor_tensor(wcr[:], wcr[:], e0[:], Alu.add)
                # rows
                r0 = atile("r0")
                vec.tensor_scalar(r0[:], dy[:], 0.0, None, Alu.is_equal)
                r1 = atile("r1")
                vec.tensor_scalar(r1[:], dy[:], 1.0, None, Alu.is_equal)
                rm = atile("rm")
                vec.tensor_scalar(rm[:], dy[:], -1.0, None, Alu.is_equal)
                wrt = atile("wrt")
                vec.tensor_tensor(wrt[:], wy0[:], r0[:], Alu.mult)
                vec.tensor_tensor(r1[:], fy[:], r1[:], Alu.mult)
                vec.tensor_tensor(wrt[:], wrt[:], r1[:], Alu.add)
                wrb = atile("wrb")
                vec.tensor_tensor(wrb[:], wy0[:], rm[:], Alu.mult)
                vec.tensor_tensor(r0[:], fy[:], r0[:], Alu.mult)
                vec.tensor_tensor(wrb[:], wrb[:], r0[:], Alu.add)
                # W4 [128, slot, tap, n] with n = (col, row):
                # 0=(L,T) 1=(L,B) 2=(R,T) 3=(R,B)
                w4 = apool.tile([128, SLOTS, KK, 4], F16, name="w4")
                vec.tensor_tensor(w4[:, :, :, 0], wcl[:], wrt[:], Alu.mult)
                vec.tensor_tensor(w4[:, :, :, 1], wcl[:], wrb[:], Alu.mult)
                vec.tensor_tensor(w4[:, :, :, 2], wcr[:], wrt[:], Alu.mult)
                vec.tensor_tensor(w4[:, :, :, 3], wcr[:], wrb[:], Alu.mult)

                # ---- idx shuffle into SWDGE wrapped-16 layout --------------
                # IDXW[q, t, s*8 + k] = idx16[16k+q, s, t]; replicated to all
                # eight 16-partition blocks.
                idxw = apool.tile([128, KK, SLOTS * 8], I16, name="idxw")
                idxwv = idxw[:].rearrange("p t (s k) -> p t s k", k=8)
                for k in range(8):
                    nc.sync.dma_start(
                        idxwv[0:16, :, :, k].squeeze(),
                        idx16[16 * k : 16 * (k + 1), :, :],
                    )
                for blk in range(1, 8):
                    nc.sync.dma_start(
                        idxw[16 * blk : 16 * (blk + 1), :, :], idxw[0:16, :, :]
                    )

                return w4, idxw

            def main_loops(w4, idxw):
                vec = nc.vector
                # gather source view: row r -> 256 contiguous floats starting
                # at r*128 (overlapping windows)
                xsrc = bass.AP(xp.tensor, 0, [[128, 2 * PIX], [1, 256]])  # fp16 rows
                gidx = [0]
                reg768 = nc.gpsimd.to_reg(768)
                reg512 = nc.gpsimd.to_reg(512)

                # ---- main loop --------------------------------------------
                for ch in range(NCHUNK):
                    spair = None
                    for t in range(KK):
                        sub = t % 2
                        pair = t // 2
                        g = gpool.tile([128, CSLOT, 4, 64], F16, name="g")
                        if "nogather" in abl and ch + t == 0:
                            nc.vector.memset(g[:], 0.25)
                        # sub-gathers sized to the SWDGE ring (<=768 descs)
                        subs = ((0, 6), (6, 6), (12, 4)) if "nogather" not in abl else ()
                        for s0, ns in subs:
                            nidx = ns * 128
                            nc.gpsimd.dma_gather(
                                g[:, s0 : s0 + ns, :, :].rearrange(
                                    "p s a c -> p s (a c)"
                                ),
                                xsrc,
                                idxw[
                                    :, t,
                                    128 * ch + 8 * s0 : 128 * ch + 8 * (s0 + ns),
                                ],
                                num_idxs=nidx,
                                num_idxs_reg=reg768 if ns == 6 else reg512,
                                elem_size=256,
                                elem_step=128,
                                queue_num=gidx[0] % 4,
                            )
                            gidx[0] += 1
                        # combine: g *= w4 (bcast over c); pairwise adds
                        if sub == 0:
                            spair = spool.tile(
                                [128, CSLOT, 2, 64], F16, name="spair"
                            )
                            if t == KK - 1 and "nodve" not in abl:
                                vec.memset(spair[:, :, 1, :], 0.0)
                        if "nodve" in abl:
                            if sub == 0:
                                vec.memset(spair[:], 0.5)
                        else:
                            wsl = (
                                w4[:, ch * CSLOT : (ch + 1) * CSLOT, t, :]
                                .unsqueeze(3)
                                .broadcast_to([128, CSLOT, 4, 64])
                            )
                            vec.tensor_tensor(g[:], g[:], wsl, Alu.mult)
                            # n=(col,row): vertical adds pairs (0,1),(2,3)
                            vec.tensor_tensor(
                                g[:, :, 0:4:2, :],
                                g[:, :, 0:4:2, :],
                                g[:, :, 1:4:2, :],
                                Alu.add,
                            )
                            vec.tensor_tensor(
                                spair[:, :, sub, :],
                                g[:, :, 0, :],
                                g[:, :, 2, :],
                                Alu.add,
                            )
                        if (sub == 1 or t == KK - 1) and "nope" not in abl:
                            # transpose pair -> channel-major, conv matmuls
                            stm = stpool.tile([128, CPIX], F16, name="stm")
                            for tb in range(CSLOT // 4):
                                pt = ptpool.tile([128, 512], F16, name="pt")
                                for j in range(4):
                                    slot = tb * 4 + j
                                    nc.tensor.matmul(
                                        pt[:, 128 * j : 128 * (j + 1)],
                                        spair[:, slot, :, :],
                                        ident[:],
                                        is_transpose=True,
                                    )
                                nc.scalar.activation(
                                    stm[:, 512 * tb : 512 * (tb + 1)],
                                    pt[:],
                                    Act.Copy,
                                )
                            if pair == 0:
                                pc = pcpool.tile([OC, CPIX], F32, name="pc")
                            for nb in range(CPIX // 512):
                                nc.tensor.matmul(
                                    pc[:, 512 * nb : 512 * (nb + 1)],
                                    wpt[:, OC * pair : OC * (pair + 1)],
                                    stm[:, 512 * nb : 512 * (nb + 1)],
                                    start=(pair == 0),
                                    stop=(pair == NPAIR - 1),
                                )
                    if "nope" in abl:
                        nc.sync.dma_start(
                            out[0:64, CPIX * ch : CPIX * ch + 1024],
                            spair[0:64, :, :, :].rearrange("p a b c -> p (a b c)")[
                                :, 0:1024
                            ],
                        )
                    else:
                        osb = opool.tile([OC, CPIX], F32, name="osb")
                        nc.scalar.activation(
                            osb[:], pc[:], Act.Identity, bias=bias[:, 0:1]
                        )
                        nc.sync.dma_start(
                            out[:, CPIX * ch : CPIX * (ch + 1)], osb[:]
                        )

            if loop_n > 0:
                with tc.For_i(0, loop_n, 1):
                    body()
            else:
                body()

    nc.compile()
    return nc


def prep_core_inputs(x, offset, weight, bias, core):
    """Host-side shard/layout prep for one core. Pure layout, no math on
    tensor values (beyond the reference-mandated reshape semantics)."""
    s, half = core // 2, core % 2
    # interleaved row-pair NHWC: xpair[par*PIX + pr*128 + j] =
    #   [x[2pr+par, j, :], x[2pr+par+1, j, :]]
    xr = np.ascontiguousarray(x[s].transpose(1, 2, 0))          # [H, W, C]
    xpad = np.concatenate([xr, np.zeros((1, W, C), np.float32)], 0)  # [129,W,C]
    even = np.stack([xpad[0::2][:64], xpad[1::2][:64]], 1)   # rows (2p, 2p+1)
    odd = np.stack([xpad[1::2][:64], xpad[2::2][:64]], 1)    # rows (2p+1, 2p+2)
    rows = np.concatenate([even, odd], 0)                    # [128, 2, W, C]
    # xpair row (par, pr, j) = [x[row_g0, j, :], x[row_g0+1, j, :]]
    xpair = np.concatenate(
        [rows.transpose(0, 2, 1, 3).reshape(2 * PIX, 128),
         np.zeros((1, 128), np.float32)], 0)

    # offsets for this half, pixel-major [PIX, 18]
    off = np.ascontiguousarray(
        offset[s, :, 64 * half : 64 * half + HALF, :]
        .transpose(1, 2, 0)
        .reshape(PIX, 2 * KK)
    )
    # channel c=2t is x-offset, 2t+1 is y-offset (reference reshape
    # [kk,2,h,w]: x = off[:,:,0], y = off[:,:,1] -> channel t*2+0 / t*2+1)

    # tap-pair weight slabs [128, 5*64]: rows sub*64+c, cols pair*64+o
    wfull = weight.reshape(OC, C, KK)
    wpair = np.zeros((128, NPAIR * OC), np.float32)
    for pair in range(NPAIR):
        for sub in range(2):
            t = pair * 2 + sub
            if t < KK:
                wpair[sub * 64 : sub * 64 + 64, pair * OC : (pair + 1) * OC] = (
                    wfull[:, :, t].T
                )

    yy = np.broadcast_to(
        (np.arange(SLOTS, dtype=np.float32) + 64 * half)[None, :], (128, SLOTS)
    ).copy()
    xxc = np.arange(128, dtype=np.float32).reshape(128, 1).copy()
    return {
        "xpair": np.ascontiguousarray(xpair).astype(np.float16),
        "offs": off.astype(np.float32),
        "wpair": wpair.astype(np.float16),
        "yy": yy,
        "xx": xxc,
        "ident": np.eye(128, dtype=np.float16),
        "bias": bias.reshape(OC, 1).astype(np.float32),
    }


_CACHE = {}


def kernel(x, offset, weight, bias):
    x = np.asarray(x, np.float32)
    offset = np.asarray(offset, np.float32)
    weight = np.asarray(weight, np.float32)
    bias = np.asarray(bias, np.float32)
    if "nc" not in _CACHE:
        _CACHE["nc"] = build_program()
    nc = _CACHE["nc"]
    in_maps = [prep_core_inputs(x, offset, weight, bias, c) for c in range(8)]
    res = run_bass_kernel_spmd(nc, in_maps, core_ids=list(range(8)))
    outf = np.empty((B, OC, H, W), np.float32)
    for c in range(8):
        s, half = c // 2, c % 2
        outf[s, :, 64 * half : 64 * half + HALF, :] = res.results[c][
            "out"
        ].reshape(OC, HALF, W)
    return outf
